# revision 1
# baseline (speedup 1.0000x reference)
"""Linear (feature-map) attention for Trainium2, 8-core head-parallel.

Math per (b,h), fp32 (s = D**-0.25):
    phi(x) = elu(s*x) + 1  ==  max(s*x, 0) + min(exp(s*x), 1)
    kv     = phi_k^T @ [v | 1]            # [64, 65]; col 64 = sum_s phi_k
    out    = (phi_q @ kv[:, :64]) / (phi_q @ kv[:, 64])

We compute with phi' = phi/s throughout; the factor cancels in the ratio.
(The reference's +1e-8 in the denominator is far below one fp32 ulp of the
~3e5-magnitude normalizer, so dropping it is bit-identical.)
The attention mask is all-ones per the input spec -> numeric no-op.

Per core: 8 of the 64 (b,h) slices, processed as 4 pairs of heads.
SBUF s-layout: s = 32*p + t (p = partition, t = 0..31) so every DMA moves
128 partitions x 8KB contiguous.

Engine plan (PE in transpose-mode + bf16 on every matmul path):
  PE  : q-transpose via transpose-mode matmul (2 cyc/row fp32), one
        [128,128] transpose per head per pair of adjacent s-tiles
        ("chunk" c covers tiles 2c, 2c+1) -> phiqT[(d_lo|d_hi), s]
        kv matmul (bf16 phi_k x bf16 [v|1], one matmul per (j,h)),
        col-tiled per head into one PSUM bank
        kv cross-half replication (tiny identity matmul, see below)
        out matmul (bf16), per head against block-diag(kv_h, kv_h)
  ACT : exp + relu for phi (bf16 out)
  DVE : fused min/add phi assembly (bf16 out), kv evac, reciprocal,
        fused normalize+evacuate (bf16 out2)
  POOL: v fp32->bf16 cast (+ ones column), out-DMA issue (keeps the sync
        queue's input-load issues free of compute waits)
The output crosses HBM in bf16 and is cast back to fp32 on the host.
Block-diag moving operand per head: msb_h[128,130] has kv_h at rows 0:64 ->
cols 0:65 and kv_h again at rows 64:128 -> cols 65:130, so one matmul
against phiqT chunk c yields out for tiles 2c (cols 0:65) and 2c+1
(cols 65:130), each with its normalizer column. kv_h lives in PSUM on one
partition half only; the other half is produced by a 65-column identity
matmul bounce (PE) + evac (DVE).
"""

import numpy as np

B, H, S_FULL, D = 4, 16, 4096, 64
N_CORES = 8
BH = B * H
BH_PER_CORE = BH // N_CORES  # 8
P = 128

SCALE = float(D) ** -0.25          # 0.3535533905932738
INV_S = 1.0 / SCALE

_NC_CACHE = {}

# Diagnostic switches (ablation benchmarks only; correctness requires all 3).
ABLATE_IN_DMA = False
ABLATE_OUT_DMA = False


def _patch_tile_drain():
    """The walrus build in this container accepts at most ONE sync wait per
    instruction, but TileContext's kernel-tail drain aggregates every
    outstanding semaphore onto a single SP Drain. Replace it with one
    single-wait SP nop per semaphore followed by the drain."""
    import concourse.mybir as mybir
    import concourse.tile as tile
    from concourse.vector_clock import ScopedClock

    if getattr(tile.TileContext, "_single_wait_drain_patch", False):
        return

    def _drain_and_barrier(self, tick_clock, wait_clock):
        collector = self.nc.sync.nop()
        wait_clock.add_sem_waits(
            collector.ins, ScopedClock({None: tick_clock.global_clock})
        )
        waits = list(collector.ins.sync_info.on_wait) if collector.ins.sync_info else []
        collector.ins.sync_info = mybir.SyncInfo(on_wait=waits[:1], on_update=[])
        for w in waits[1:]:
            nop = self.nc.sync.nop()
            nop.ins.sync_info = mybir.SyncInfo(on_wait=[w], on_update=[])
        self.nc.sync.drain()
        self.nc.all_engine_barrier()
        assert self.sems is not None
        popped = self.nc._tile_sem_poison_stack.pop()
        assert popped is self._sem_poison
        self.nc.clear_and_free_semaphores(list(self.sems.allocated().values()))
        self.nc.all_engine_barrier()

    tile.TileContext._drain_and_barrier = _drain_and_barrier

    # General wait-splitting: any scheduled instruction that ends up with
    # more than one sync wait gets single-wait NoOps injected in front of it
    # on the same engine stream (semantically identical synchronization).
    _orig_commit = tile.TileContext._commit_instruction

    def _commit_instruction(self, inst, lazy_reg_writes=True):
        si = getattr(inst, "sync_info", None)
        if si is not None and si.on_wait and len(si.on_wait) > 1:
            waits = list(si.on_wait)
            for w in waits[:-1]:
                nop = mybir.InstNoOp(
                    name=self.nc.get_next_instruction_name(),
                    engine=inst.engine,
                    text_hint="wait_split",
                    bass_nofuse=True,
                )
                nop.sync_info = mybir.SyncInfo(on_wait=[w], on_update=[])
                _orig_commit(self, nop, lazy_reg_writes)
            inst.sync_info = mybir.SyncInfo(
                on_wait=[waits[-1]], on_update=list(si.on_update or [])
            )
        return _orig_commit(self, inst, lazy_reg_writes)

    tile.TileContext._commit_instruction = _commit_instruction
    tile.TileContext._single_wait_drain_patch = True


def build_bass(n_heads=BH_PER_CORE, S=S_FULL, n_reps=1, hw_loop=False):
    """hw_loop: emit the body once inside a tc.For_i hardware loop running
    n_reps iterations (for timing: keeps the NEFF small so large n_reps does
    not hit instruction-stream footprint effects)."""
    import concourse.bass as bass
    import concourse.mybir as mybir
    import concourse.tile as tile

    _patch_tile_drain()

    f32 = mybir.dt.float32
    nc = bass.Bass("TRN2")
    q_d = nc.dram_tensor("q", [n_heads, S, D], f32, kind="ExternalInput")
    k_d = nc.dram_tensor("k", [n_heads, S, D], f32, kind="ExternalInput")
    v_d = nc.dram_tensor("v", [n_heads, S, D], f32, kind="ExternalInput")
    bf16 = mybir.dt.bfloat16
    # out leaves the device in bf16 (host casts back to fp32): halves the
    # out-DMA HBM traffic; well within the 2e-2 tolerance.
    o_d = nc.dram_tensor("out", [n_heads, S, D], bf16, kind="ExternalOutput")
    with tile.TileContext(nc) as tc:
        if hw_loop and n_reps > 1:
            with tc.For_i(0, n_reps):
                _emit(tc, q_d, k_d, v_d, o_d, n_heads, S, 1)
        else:
            _emit(tc, q_d, k_d, v_d, o_d, n_heads, S, n_reps)
    nc.finalize()
    return nc


def _emit(tc, q_d, k_d, v_d, o_d, n_heads, S, n_reps=1):
    from contextlib import ExitStack

    import concourse.mybir as mybir
    from concourse.masks import make_identity

    nc = tc.nc
    f32 = mybir.dt.float32
    bf16 = mybir.dt.bfloat16
    Alu = mybir.AluOpType
    Act = mybir.ActivationFunctionType

    T = S // P                # s-tiles per head (32 for S=4096)
    n_pairs = n_heads // 2

    ctx = ExitStack()
    with ctx:
        p_const = ctx.enter_context(tc.tile_pool(name="const", bufs=1))
        p_qin = ctx.enter_context(tc.tile_pool(name="qin", bufs=2))
        p_kin = ctx.enter_context(tc.tile_pool(name="kin", bufs=2))
        p_vin = ctx.enter_context(tc.tile_pool(name="vin", bufs=2))
        p_vb = ctx.enter_context(tc.tile_pool(name="vb", bufs=2))
        p_ek = ctx.enter_context(tc.tile_pool(name="ek", bufs=2))
        p_rk = ctx.enter_context(tc.tile_pool(name="rk", bufs=2))
        p_mk = ctx.enter_context(tc.tile_pool(name="mk", bufs=2))
        p_eq = ctx.enter_context(tc.tile_pool(name="eq", bufs=2))
        p_phiqt = ctx.enter_context(tc.tile_pool(name="phiqt", bufs=2))
        p_small = ctx.enter_context(tc.tile_pool(name="small", bufs=2))
        p_out = ctx.enter_context(tc.tile_pool(name="outb", bufs=2))
        ps_qt = ctx.enter_context(tc.tile_pool(name="psqt", bufs=2, space="PSUM"))
        ps_kv = ctx.enter_context(tc.tile_pool(name="pskv", bufs=1, space="PSUM"))
        ps_bq = ctx.enter_context(tc.tile_pool(name="psbq", bufs=1, space="PSUM"))
        ps_o = ctx.enter_context(tc.tile_pool(name="pso", bufs=2, space="PSUM"))
        ident = p_const.tile([P, P], f32, tag="ident")
        make_identity(nc, ident[:])
        identb = p_const.tile([P, P], bf16, tag="identb")
        nc.vector.tensor_copy(identb[:], ident[:])
        ones = p_const.tile([P, 1], f32, tag="ones")
        nc.vector.memset(ones[:], 1.0)
        onesb = p_const.tile([P, 1], bf16, tag="onesb")
        nc.vector.memset(onesb[:], 1.0)
        # engine progress markers for semaphore priming (see _emit_body).
        # Each marker tile has exactly one writer engine and one reader
        # engine so marker writes themselves never need two waits.
        ascr = p_const.tile([1, 2], f32, tag="ascr")    # ACT writes, none read
        dscr = p_const.tile([1, 2], f32, tag="dscr")    # DVE writes, none read
        m_ap = p_const.tile([1, 1], f32, tag="m_ap")    # ACT -> PE
        m_dp = p_const.tile([1, 1], f32, tag="m_dp")    # DVE -> PE
        m_da = p_const.tile([1, 1], f32, tag="m_da")    # DVE -> ACT
        m_ad = p_const.tile([1, 1], f32, tag="m_ad")    # ACT -> DVE
        nc.vector.tensor_copy(m_da[0:1, 0:1], ones[0:1, 0:1])
        nc.scalar.copy(m_ad[0:1, 0:1], ones[0:1, 0:1])
        st = {"prev_kvone1": None}
        for _rep in range(n_reps):
            _emit_body(
                nc, mybir, f32, Alu, Act, T, n_pairs,
                p_qin, p_kin, p_vin, p_vb, p_ek, p_rk, p_mk, p_eq, p_phiqt,
                p_small,
                p_out, ps_qt, ps_kv, ps_bq, ps_o, q_d, k_d, v_d, o_d,
                ident, identb, ones, onesb, ascr, dscr, m_ap, m_dp, m_da,
                m_ad, st,
            )


def _emit_body(
    nc, mybir, f32, Alu, Act, T, n_pairs,
    p_qin, p_kin, p_vin, p_vb, p_ek, p_rk, p_mk, p_eq, p_phiqt, p_small, p_out,
    ps_qt, ps_kv, ps_bq, ps_o, q_d, k_d, v_d, o_d,
    ident, identb, ones, onesb, ascr, dscr, m_ap, m_dp, m_da, m_ad, st,
):
    bf16 = mybir.dt.bfloat16
    # The TRN2 ISA allows at most ONE semaphore wait per engine instruction,
    # and Tile attaches a wait for every fresh cross-engine dependency
    # (including same-engine write-after-read completions). Any instruction
    # with >=2 fresh dependencies fails codegen with "Too many sync wait
    # commands". Throughout this body, tiny single-dependency "observer"
    # instructions advance each engine's semaphore view one step at a time so
    # every real instruction needs at most one wait (the _patch_tile_drain
    # wait-splitter catches any leftovers). phi is computed as
    #   phi(s*x) = Relu(s*x) + min(Exp(s*x), 1)
    # with the scale folded into the ACT ops, so no engine ever needs a
    # separate scale pass.

    NC_ = T // 2  # transpose chunks per head (16); chunk c = s-tiles 2c, 2c+1

    for pr in range(n_pairs):
        iA, iB = 2 * pr, 2 * pr + 1

        # ---- loads: s = 32*p + t layout, 8KB contiguous per partition ----
        q2 = p_qin.tile([P, 2, T, D], f32, tag="q2")
        k2 = p_kin.tile([P, 2, T, D], f32, tag="k2")
        v2 = p_vin.tile([P, 2, T, D], f32, tag="v2")
        for t_s, t_d in ((q2, q_d), (k2, k_d), (v2, v_d)):
            if ABLATE_IN_DMA:
                # tiny loads: keep tiles allocated/written, ~3% of the bytes
                for h, i in ((0, iA), (1, iB)):
                    nc.sync.dma_start(
                        t_s[:, h, 0:1, :],
                        t_d[i].rearrange("(p t) d -> p t d", p=P)[:, 0:1, :],
                    )
                continue
            # both heads in one start: dram [2, S, D] contiguous, 8KB chunks
            nc.sync.dma_start(
                t_s[:],
                t_d[iA : iB + 1].rearrange("h (p t) d -> p h t d", p=P),
            )

        # ---- k path: ek = Exp(s*k); rk = Relu(s*k);
        #      ek <- min(ek,1) + rk  (= phi_k, consumed by mm1) -------------
        # v cast fp32 -> bf16 on the (otherwise idle) GPSIMD engine so
        # mm1 runs at bf16 rate.
        vb = p_vb.tile([P, 2, T, D + 1], bf16, tag="vb")
        nc.gpsimd.tensor_copy(vb[:, :, :, 0:D], v2[:])
        nc.gpsimd.memset(vb[:, :, :, D : D + 1], 1.0)
        # phi_k = min(Exp(s*k), 1) + Relu(s*k), assembled by one fused DVE
        # stt per chunk; everything bf16 so mm1 runs at bf16 rate.
        ek = p_ek.tile([P, 2, T, D], bf16, tag="ek")
        rk = p_rk.tile([P, 2, T, D], bf16, tag="rk")
        # ACT observers: prior readers of this ek buffer (PE via last pair's
        # mm1 weight loads, DVE via the stt) + the two k2 DMA lanes.
        if st["prev_kvone1"] is not None:
            nc.scalar.copy(ascr[0:1, 0:1], st["prev_kvone1"][64:65, 0:1])
            nc.scalar.copy(ascr[0:1, 1:2], m_da[0:1, 0:1])
        nc.scalar.copy(ek[0:1, 0, 0, 0:1], ones[0:1, 0:1])
        nc.scalar.copy(ek[0:1, 0, 0, 1:2], k2[0:1, 0, 0, 0:1])
        nc.scalar.copy(ek[0:1, 0, 0, 2:3], k2[0:1, 1, 0, 0:1])
        kch = max(T // 2, 1)
        for c0 in range(0, T, kch):
            sl = slice(c0, c0 + kch)
            nc.scalar.activation(ek[:, :, sl, :], k2[:, :, sl, :], Act.Exp, scale=SCALE)
            nc.scalar.activation(rk[:, :, sl, :], k2[:, :, sl, :], Act.Relu, scale=SCALE)
            if c0 == 0:
                # ACT->PE marker; reads an exp output so its tick covers the
                # exp (markers need a data dependency or the ready-first
                # scheduler runs them before the work they mark)
                nc.scalar.copy(m_ap[0:1, 0:1], ek[0:1, 0, 0, 3:4])
            # phi_k -> ek (DVE: one fused op; reads only ACT-produced data)
            nc.vector.scalar_tensor_tensor(
                ek[:, :, sl, :], ek[:, :, sl, :], 1.0, rk[:, :, sl, :],
                Alu.min, Alu.add,
            )
            if c0 == 0:
                # DVE->PE and DVE->ACT markers, data-dependent on the stt
                nc.vector.tensor_copy(m_dp[0:1, 0:1], ek[0:1, 0, 0, 0:1])
                nc.vector.tensor_copy(m_da[0:1, 0:1], ek[0:1, 0, 0, 1:2])

        # ---- PSUM accumulators --------------------------------------------
        # kvx[h] bank: cols 0:64 = kv_h, col 64 = k_one_h. Both accumulation
        # series share the bank: the first kv matmul's start=True clears
        # has_written bank-wide, so the k_one series runs start=False and
        # still overwrites on its first j (per-element has_written drives
        # add-vs-overwrite).
        kvx = [
            ps_kv.tile([P, 65], f32, tag=f"kvx{h}", name=f"kvx{h}") for h in (0, 1)
        ]
        # PE observers (before the transposes): the kvx[0] bank release
        # (DVE), then the two q2 DMA lanes. All write the same psum element,
        # which the later start=True accumulation overwrites.
        nc.tensor.matmul(kvx[0][0:1, 0:1], ones[0:1, 0:1], ones[0:1, 0:1])
        nc.tensor.matmul(kvx[0][0:1, 0:1], q2[0:1, 0, 0, 0:1], q2[0:1, 0, 0, 0:1])
        nc.tensor.matmul(kvx[0][0:1, 0:1], q2[0:1, 1, 0, 0:1], q2[0:1, 1, 0, 0:1])

        # ---- q path: PE transpose-mode -> phi_q in transposed layout ------
        # phiqT[:, h, c, :]: partitions = (d of tile 2c | d of tile 2c+1),
        # free = the 128 s of the corresponding tile
        phiqT = p_phiqt.tile([P, 2, NC_, P], bf16, tag="phiqt")
        # DVE observer: phiqT buffer release (PE read it last pair)
        nc.vector.tensor_copy(phiqT[0:1, 0, 0, 0:1], ones[0:1, 0:1])
        for h in (0, 1):
            for cb in range(NC_ // 4):
                qtp = ps_qt.tile([P, 4, P], f32, tag="qtp")
                for cc in range(4):
                    c = 4 * cb + cc
                    # transpose-mode matmul (2 cyc/row fp32): [128s, 128d']
                    # -> [128d', 128s] with d' = (d of tile 2c | tile 2c+1);
                    # output must start at PSUM partition 0 in this mode.
                    nc.tensor.matmul(
                        qtp[:, cc, :],
                        q2[:, h, 2 * c : 2 * c + 2, :].rearrange(
                            "p a b -> p (a b)"
                        ),
                        ident[:],
                        is_transpose=True,
                    )
                # ACT is the only reader of the qtp bank (exp AND relu), so
                # the bank release back to PE is a single semaphore. ACT
                # observers: the eq/rq buffer release (DVE stt), then this
                # bank's PE tick.
                eq = p_eq.tile([P, 4, P], bf16, tag="eq")
                rq = p_eq.tile([P, 4, P], bf16, tag="rq")
                nc.scalar.copy(eq[0:1, 0, 0:1], ones[0:1, 0:1])
                nc.scalar.copy(eq[0:1, 0, 1:2], qtp[0:1, 0, 0:1])
                nc.scalar.activation(eq[:], qtp[:], Act.Exp, scale=SCALE)
                nc.scalar.activation(rq[:], qtp[:], Act.Relu, scale=SCALE)
                # phi_q = min(Exp,1) + Relu  (one fused DVE op per bank)
                nc.vector.scalar_tensor_tensor(
                    phiqT[:, h, 4 * cb : 4 * cb + 4, :],
                    eq[:],
                    1.0,
                    rq[:],
                    Alu.min,
                    Alu.add,
                )

        # PE observers (before mm1): the two v2 DMA lanes and the DVE tick of
        # the finished phi_k (ek) write.
        nc.tensor.matmul(kvx[0][0:1, 0:1], vb[0:1, 0, 0, 0:1], vb[0:1, 0, 0, 0:1])
        nc.tensor.matmul(kvx[0][0:1, 0:1], vb[0:1, 1, 0, 0:1], vb[0:1, 1, 0, 0:1])
        nc.tensor.matmul(kvx[0][0:1, 0:1], m_ap[0:1, 0:1], m_ap[0:1, 0:1])
        nc.tensor.matmul(kvx[0][0:1, 0:1], m_dp[0:1, 0:1], m_dp[0:1, 0:1])

        # ---- kv = phi_k^T @ v and k_one = phi_k^T @ 1 ---------------------
        # Head h's output sits at partitions 64h..64h+63 (col-tiled, the two
        # heads' matmuls run concurrently on PE).
        # vb col 64 is constant 1.0, so one matmul per (j, h) yields both kv
        # (cols 0:64) and k_one (col 64)
        for j in range(T):
            sta, sp = (j == 0), (j == T - 1)
            for h in (0, 1):
                nc.tensor.matmul(
                    kvx[h][64 * h : 64 * h + 64, :], ek[:, h, j, :],
                    vb[:, h, j, :], start=sta, stop=sp,
                )

        # ---- per-head block-diag moving operand for mm2 -------------------
        # msb[:, h]: [128, 130] bf16 = blockdiag(kv_h|kone_h ; kv_h|kone_h).
        # Direct half straight from PSUM; the other half bounces through a
        # 65-column identity matmul (PE) since DVE cannot shift partitions.
        msb = p_small.tile([P, 2, 130], bf16, tag="msb")
        nc.vector.memset(msb[:], 0.0)
        bq = [None, None]
        for h in (0, 1):
            lo = 64 * h
            dcols = slice(0, 65) if h == 0 else slice(65, 130)
            ocols = slice(65, 130) if h == 0 else slice(0, 65)
            # direct half: [kv_h | k_one_h] -> its matching block
            nc.vector.tensor_copy(
                msb[lo : lo + 64, h, dcols], kvx[h][lo : lo + 64, :]
            )
            # DVE->PE marker covering the direct copy (single dep each)
            nc.vector.tensor_copy(
                m_dp[0:1, 0:1], msb[lo : lo + 1, h, dcols.start : dcols.start + 1]
            )
            # PE: replicate to the other partition half via identity matmul
            bq[h] = ps_bq.tile([P, 65], f32, tag=f"bq{h}", name=f"bq{h}")
            # PE observer: the bq bank release (DVE evac two pairs ago) and
            # the m_dp tick above
            nc.tensor.matmul(bq[h][0:1, 0:1], ones[0:1, 0:1], ones[0:1, 0:1])
            nc.tensor.matmul(bq[h][0:1, 0:1], m_dp[0:1, 0:1], m_dp[0:1, 0:1])
            other = 64 - lo
            nc.tensor.matmul(
                bq[h][other : other + 64, :],
                identb[lo : lo + 64, lo : lo + 64],
                msb[lo : lo + 64, h, dcols],
            )
            # DVE: evacuate the replicated half into the other block
            nc.vector.tensor_copy(
                msb[other : other + 64, h, ocols], bq[h][other : other + 64, :]
            )

        # ---- out = phi_q @ kv ; fused normalize + evacuate ---------------
        out2 = p_out.tile([P, 2, T, D], bf16, tag="out2")
        rc = p_small.tile([P, 2, NC_, 2], f32, tag="recip")
        # DVE observers: the two out-DMA lanes that released this out2 buffer
        nc.vector.tensor_copy(out2[0:1, 0, 0, 0:1], ones[0:1, 0:1])
        nc.vector.tensor_copy(out2[0:1, 1, 0, 0:1], ones[0:1, 0:1])
        n_ob = (NC_ + 2) // 3
        for h, i in ((0, iA), (1, iB)):
            for m in range(n_ob):
                w = min(3, NC_ - 3 * m)
                op = ps_o.tile([P, 3, 130], f32, tag="op")
                for cc in range(w):
                    c = 3 * m + cc
                    # [128s, 130]: cols 0-64 tile 2c (col 64 = norm),
                    # cols 65-129 tile 2c+1
                    nc.tensor.matmul(op[:, cc, :], phiqT[:, h, c, :], msb[:, h, :])
                opv = op[:, 0:w, :].rearrange("p j (b e) -> p j b e", b=2)
                nc.vector.reciprocal(
                    rc[:, h, 3 * m : 3 * m + w, :], opv[:, :, :, 64]
                )
                nc.vector.tensor_tensor(
                    out2[:, h, 6 * m : 6 * m + 2 * w, :].rearrange(
                        "p (j b) e -> p j b e", b=2
                    ),
                    opv[:, :, :, 0:64],
                    rc[:, h, 3 * m : 3 * m + w, :, None].to_broadcast(
                        (P, w, 2, D)
                    ),
                    Alu.mult,
                )
                if 3 * m + w == 9 and not ABLATE_OUT_DMA:
                    # first-half out-DMA leaves while mm2 finishes the rest.
                    # Issued from the (otherwise idle) GPSIMD queue: an
                    # out-DMA start waits on the DVE normalize, and on the
                    # sync queue that wait would head-of-line-block the next
                    # pair's input-load issues, serializing DMA vs compute.
                    od = o_d[i].rearrange("(p t) d -> p t d", p=P)
                    nc.gpsimd.dma_start(od[:, :18, :], out2[:, h, :18, :])
            if not ABLATE_OUT_DMA:
                od = o_d[i].rearrange("(p t) d -> p t d", p=P)
                nc.gpsimd.dma_start(od[:, 18:, :], out2[:, h, 18:, :])

        st["prev_kvone1"] = kvx[1]


def _get_nc():
    key = (BH_PER_CORE, S_FULL)
    if key not in _NC_CACHE:
        _NC_CACHE[key] = build_bass(*key)
    return _NC_CACHE[key]


def run_sharded(q, k, v, trace=False):
    """q/k/v: [BH, S, D] fp32 numpy. Returns ([BH, S, D] fp32, BassKernelResults)."""
    from concourse.bass_utils import run_bass_kernel_spmd

    nc = _get_nc()
    in_maps = []
    for c in range(N_CORES):
        sl = slice(c * BH_PER_CORE, (c + 1) * BH_PER_CORE)
        in_maps.append(
            {
                "q": np.ascontiguousarray(q[sl]),
                "k": np.ascontiguousarray(k[sl]),
                "v": np.ascontiguousarray(v[sl]),
            }
        )
    res = run_bass_kernel_spmd(
        nc, in_maps, core_ids=list(range(N_CORES)), trace=trace
    )
    out = np.concatenate(
        [np.asarray(r["out"], dtype=np.float32) for r in res.results], axis=0
    )
    return out, res


def kernel(query, key, value, attention_mask=None):
    q = np.asarray(query, dtype=np.float32).reshape(BH, S_FULL, D)
    k = np.asarray(key, dtype=np.float32).reshape(BH, S_FULL, D)
    v = np.asarray(value, dtype=np.float32).reshape(BH, S_FULL, D)
    out, _ = run_sharded(q, k, v, trace=False)
    return out.reshape(B, H, S_FULL, D)

